# revision 14
# baseline (speedup 1.0000x reference)
"""Causal multi-head attention (B=4, S=2048, D=1024, H=16) on 8 TRN2 NeuronCores.

Sharding: 4 batches x 2 head-groups (8 heads each) -> 8 cores.

v2 pipeline (vs v1 baseline at 766us):
  - qc-outer attention loop; per-qc output projection + chunked
    ReduceScatter overlapped with the next chunk's attention.
  - QK^T matmuls for the odd head run directly on partitions 64:128
    (tile_position) -- no DVE staging copies.
  - exp scores and V in bf16: PV matmuls at full PE rate, DVE 2x mask ops.
  - diag-block QK/exp shrunk to the unmasked moving range; masked PV
    columns stay zero via pre-zeroed persistent diag et tiles.
  - softmax normalization: reciprocal_approx_fast on the [1,512] denom row,
    K=1 ones-outer-product matmul broadcasts it across partitions, one DVE
    mul writes normalized ctx^T -- no DRAM round trip, no slow reciprocal.
  - deferred-emission queue interleaves normalize/outproj/collective work
    into the next chunk's kb loop to keep the PE array continuously busy
    (p-state ramp: the PE only reaches 2.4 GHz after ~3us without gaps).
"""

import numpy as np

B, S, D = 4, 2048, 1024
H = 16
HD = D // H  # 64
G = 2  # head groups (tensor-parallel degree per batch)
HPG = H // G  # 8 heads per core
DG = D // G  # 512 dims per group
P = 128
NKT = D // P  # 8 k-tiles over d_model
NQC = S // 512  # 4 query chunks of 512
NTT = S // P  # 16 token tiles of 128
NR = DG // P  # 4 dim-tiles (head pairs) per group
SPLIT = S // (2 * NQC)  # 256 tokens per core per chunk after RS

# moving-range start per diagonal block s = kb - 4*qc (block fully masked
# below this column; the lone +1 element keeps s>=1 blocks one subblock wider)
Q_START = [0, 0, 128, 256, 256]
# mask multiply range per s (elem-column subblock + triangular subblock)
M_RANGE = [(0, 128), (0, 256), (128, 384), (256, 512), (256, 512)]

_CACHE = {}


def _build_masks():
    """masks[s] is the [128, 512] multiplicative mask for a scoresT block
    [k_local, q_chunk_local] whose k-block index is kb = 4*qc + s.
    Allowed iff global k <= global q + 1."""
    masks = np.zeros((5, P, 512), dtype=np.float32)
    i = np.arange(P)[:, None]  # k local
    jj = np.arange(P)[None, :]  # q local within 128-subblock
    for s in range(5):
        for j in range(4):  # q subblock within the 512 chunk
            blk = masks[s][:, 128 * j : 128 * (j + 1)]
            if j > s:
                blk[:] = 1.0
            elif j == s:
                blk[:] = (i <= jj + 1).astype(np.float32)
            elif j == s - 1:
                blk[0, 127] = 1.0
    return masks


def _build_bass():
    import concourse.bacc as bacc
    import concourse.mybir as mybir
    import concourse.tile as tile

    f32 = mybir.dt.float32
    f32r = mybir.dt.float32r
    bf16 = mybir.dt.bfloat16
    AF = mybir.ActivationFunctionType

    nc = bacc.Bacc("TRN2", target_bir_lowering=False, debug=False, num_devices=8)

    xT = nc.dram_tensor("xT", [D, S], f32r, kind="ExternalInput").ap()
    wq = nc.dram_tensor("wq", [D, DG], f32r, kind="ExternalInput").ap()
    wk = nc.dram_tensor("wk", [D, DG], f32r, kind="ExternalInput").ap()
    wv = nc.dram_tensor("wv", [D, DG], f32r, kind="ExternalInput").ap()
    wo = nc.dram_tensor("wo", [DG, D], f32r, kind="ExternalInput").ap()
    bo_b = nc.dram_tensor("bo_b", [P, D], f32, kind="ExternalInput").ap()
    masks = nc.dram_tensor("masks", [5, P, 512], f32, kind="ExternalInput").ap()
    ones_r = nc.dram_tensor("ones_r", [1, 64], f32r, kind="ExternalInput").ap()
    out_ext = nc.dram_tensor("out", [NQC, SPLIT, D], f32, kind="ExternalOutput").ap()

    with tile.TileContext(nc) as tc:
        with (
            tc.tile_pool(name="pqk", bufs=1) as pqk,
            tc.tile_pool(name="pv", bufs=1) as pv,
            tc.tile_pool(name="pmask", bufs=1) as pmask,
            tc.tile_pool(name="pdram", bufs=1, space="DRAM") as pdram,
        ):
            # persistent SBUF tensors
            qT_sb = pqk.tile([P, NR, S], f32r)  # [dims of pair r | token]
            kT_sb = pqk.tile([P, NR, S], f32r)
            va_sb = pv.tile([P, NTT, HPG, HD + 1], bf16)  # v + ones col
            masks32_sb = pmask.tile([P, 5, 512], f32)
            masks_sb = pmask.tile([P, 5, 512], bf16)
            ones_sb = pmask.tile([1, 64], f32r)

            # ---------------- projections ----------------
            with (
                tc.tile_pool(name="pw", bufs=3) as pw,
                tc.tile_pool(name="px", bufs=2) as px,
                tc.tile_pool(name="pp", bufs=4, space="PSUM") as pp,
            ):
                xT_r = xT.rearrange("(ko p) t -> p ko t", p=P)
                # wq + the first x tile are split per-kt so the first matmul
                # only waits on two small transfers, not the full input queue
                w_sbs = {}
                w_sb = pw.tile([P, NKT, DG], f32r, name="w_wq", tag="w")
                wq_r = wq.rearrange("(ko p) f -> p ko f", p=P)
                xtile0 = px.tile([P, NKT, 512], f32r, name="xtile", tag="x")
                for kt in range(NKT):
                    nc.sync.dma_start(w_sb[:, kt, :], wq_r[:, kt, :])
                    nc.sync.dma_start(xtile0[:, kt, :], xT_r[:, kt, 0:512])
                w_sbs["wq"] = w_sb
                for name, w in (("wk", wk), ("wv", wv)):
                    w_sb = pw.tile([P, NKT, DG], f32r, name=f"w_{name}", tag="w")
                    nc.sync.dma_start(w_sb[:], w.rearrange("(ko p) f -> p ko f", p=P))
                    w_sbs[name] = w_sb
                # not needed until attention starts; keep off the critical path
                nc.sync.dma_start(masks32_sb[:], masks.rearrange("s p q -> p s q"))
                nc.vector.tensor_copy(masks_sb[:], masks32_sb[:])
                nc.sync.dma_start(ones_sb[:], ones_r)
                nc.vector.memset(va_sb[:, :, :, HD : HD + 1], 1.0)

                xtiles = {0: xtile0}
                for t in range(NQC):
                    tok = slice(512 * t, 512 * (t + 1))
                    xtile = xtiles.pop(t)
                    if t + 1 < NQC:
                        nxt = px.tile([P, NKT, 512], f32r, name="xtile", tag="x")
                        nc.sync.dma_start(
                            nxt[:], xT_r[:, :, slice(512 * (t + 1), 512 * (t + 2))]
                        )
                        xtiles[t + 1] = nxt
                    # qT / kT: out [dims(pair r), 512 tokens]
                    for name, dst in (("wq", qT_sb), ("wk", kT_sb)):
                        w_sb = w_sbs[name]
                        for rr in range(NR):
                            ps = pp.tile([P, 512], f32, name="ps_proj", tag="ps")
                            for kt in range(NKT):
                                nc.tensor.matmul(
                                    ps[:],
                                    w_sb[:, kt, P * rr : P * (rr + 1)],
                                    xtile[:, kt, :],
                                    start=(kt == 0),
                                    stop=(kt == NKT - 1),
                                )
                            nc.vector.tensor_copy(dst[:, rr, tok], ps[:])
                    # v: out [128 tokens, 512 dims] per token tile -> bf16
                    w_sb = w_sbs["wv"]
                    for st in range(4):
                        tt = 4 * t + st
                        ps = pp.tile([P, 512], f32, name="ps_v", tag="ps")
                        for kt in range(NKT):
                            nc.tensor.matmul(
                                ps[:],
                                xtile[:, kt, 128 * st : 128 * (st + 1)],
                                w_sb[:, kt, :],
                                start=(kt == 0),
                                stop=(kt == NKT - 1),
                            )
                        nc.vector.tensor_copy(
                            va_sb[:, tt, :, 0:HD],
                            ps[:].rearrange("p (h d) -> p h d", d=HD),
                        )

            # ---------------- attention + chunked output projection ----------------
            with (
                tc.tile_pool(name="pw2", bufs=1) as pw2,
                tc.tile_pool(name="pc", bufs=2) as pc,
                tc.tile_pool(name="pdg", bufs=1) as pdg,
                tc.tile_pool(name="pe", bufs=4) as pe,
                tc.tile_pool(name="pn", bufs=2) as pn,
                tc.tile_pool(name="po_sb", bufs=2) as po_sb,
                tc.tile_pool(name="psS", bufs=2, space="PSUM") as psS,
                tc.tile_pool(name="psC", bufs=4, space="PSUM") as psC,
            ):
                wo_sb = pw2.tile([P, NR, D], f32r)
                nc.sync.dma_start(wo_sb[:], wo.rearrange("(ko p) f -> p ko f", p=P))
                bo_sb = pw2.tile([P, D], f32)
                nc.sync.dma_start(bo_sb[:], bo_b[:])

                # persistent diag et tiles; never-written prefix stays zero
                et_d = []
                for s in range(5):
                    td = pdg.tile([P, 2, 512], bf16, name=f"et_d{s}", tag=f"et_d{s}")
                    if Q_START[s] > 0:
                        nc.vector.memset(td[:, :, 0 : Q_START[s]], 0)
                    et_d.append(td)

                # deferred-emission queue: one thunk popped per kb iteration
                # (normalize lags its pr by ~1 block; outproj/RS of chunk qc
                # interleave into chunk qc+1's attention)
                pending = []

                def pop_one():
                    if pending:
                        pending.pop(0)()

                def emit_normalize(ctx0, ctx1, ctxT, pr):
                    # Inline: denominator rows -> reciprocal -> f32r cast.
                    # All DVE, no DMA anywhere in the normalize chain (a DMA
                    # bounce here gets stuck behind ReduceScatter transfers
                    # and head-of-line blocks the DVE queue for ~10-30us).
                    # (custom-DVE reciprocal misbehaves on PSUM inputs;
                    # stage the denominator rows through SBUF first)
                    dr = pn.tile([1, 2, 512], f32, name="dr", tag="dr")
                    nc.vector.tensor_copy(dr[:, 0, :], ctx0[HD : HD + 1, :])
                    nc.vector.tensor_copy(dr[:, 1, :], ctx1[HD : HD + 1, :])
                    rrc = pn.tile([1, 2, 512], f32, name="rrc", tag="rr")
                    nc.vector.reciprocal_approx_fast(rrc[:, 0, :], dr[:, 0, :])
                    nc.vector.reciprocal_approx_fast(rrc[:, 1, :], dr[:, 1, :])
                    rrr = pn.tile([1, 2, 512], f32r, name="rrr", tag="rrr")
                    nc.vector.tensor_copy(rrr[:], rrc[:])

                    def bcast_thunk():
                        # ones-outer-product broadcast on the PE (f32r,
                        # 1 cyc/row); lands between QK/PV matmuls
                        bc = psS.tile([P, 2, 512], f32, name="bc", tag="sc")
                        nc.tensor.matmul(
                            bc[0:HD, 0, :], ones_sb[:], rrr[:, 0, :],
                            start=True, stop=True,
                        )
                        nc.tensor.matmul(
                            bc[0:HD, 1, :], ones_sb[:], rrr[:, 1, :],
                            start=True, stop=True,
                        )

                        def mul_thunk():
                            bc_sb = pn.tile(
                                [HD, 2, 512], f32, name="bc_sb", tag="bc_sb"
                            )
                            nc.vector.tensor_copy(bc_sb[:], bc[0:HD, :, :])
                            nc.vector.tensor_mul(
                                ctxT[0:HD, pr, :], ctx0[0:HD, :], bc_sb[:, 0, :]
                            )
                            nc.vector.tensor_mul(
                                ctxT[HD:P, pr, :], ctx1[0:HD, :], bc_sb[:, 1, :]
                            )

                        pending.insert(0, mul_thunk)

                    # spacers: the DVE recip/cast chain (~3us) must complete
                    # before the bcast matmul reaches the head of the PE queue
                    pending.append(lambda: None)
                    pending.append(lambda: None)
                    pending.append(bcast_thunk)

                def emit_outproj(ctxT, qc):
                    partial_d = pdram.tile(
                        [512, D], f32, name="partial", tag="partial", bufs=2
                    )
                    rs_d = pdram.tile([SPLIT, D], f32, name="rs", tag="rs", bufs=2)
                    for tt_l in range(4):
                        ts_ = slice(128 * tt_l, 128 * (tt_l + 1))
                        for nch in range(2):
                            ns = slice(512 * nch, 512 * (nch + 1))

                            def thunk(ts_=ts_, ns=ns):
                                ps = psS.tile([P, 512], f32, name="ps_o", tag="sc")
                                for rr in range(NR):
                                    nc.tensor.matmul(
                                        ps[:],
                                        ctxT[:, rr, ts_],
                                        wo_sb[:, rr, ns],
                                        start=(rr == 0),
                                        stop=(rr == NR - 1),
                                    )
                                ot = po_sb.tile([P, 512], f32, name="ot", tag="ot")
                                nc.vector.tensor_add(ot[:], ps[:], bo_sb[:, ns])
                                nc.sync.dma_start(partial_d[ts_, ns], ot[:])

                            pending.append(thunk)

                    def cc_thunk():
                        nc.gpsimd.collective_compute(
                            "ReduceScatter",
                            mybir.AluOpType.add,
                            replica_groups=[[0, 1], [2, 3], [4, 5], [6, 7]],
                            ins=[partial_d.opt()],
                            outs=[rs_d.opt()],
                        )
                        # issue from the gpsimd queue: on the SP queue this
                        # DMA's wait-for-RS head-of-line blocks every later
                        # DMA; split into 4 so the chunks ride parallel rings
                        for c4 in range(4):
                            rsl = slice(64 * c4, 64 * (c4 + 1))
                            nc.gpsimd.dma_start(out_ext[qc][rsl, :], rs_d[rsl, :])

                    pending.append(cc_thunk)

                for qc in range(NQC):
                    qs = slice(512 * qc, 512 * (qc + 1))
                    nkb = min(4 * qc + 5, NTT)
                    ctxT = pc.tile([P, NR, 512], f32r, name="ctxT", tag="ctxT")
                    for pr in range(NR):
                        ctx0 = psC.tile([HD + 1, 512], f32, name="ctx0", tag="ctx")
                        ctx1 = psC.tile([HD + 1, 512], f32, name="ctx1", tag="ctx")
                        for kb in range(nkb):
                            if kb >= 1:
                                pop_one()
                            s = kb - 4 * qc
                            diag = s >= 0
                            qst = Q_START[s] if diag else 0
                            ks = slice(128 * kb, 128 * (kb + 1))
                            qsl = slice(512 * qc + qst, 512 * (qc + 1))
                            if diag:
                                et = et_d[s]
                            else:
                                et = pe.tile([P, 2, 512], bf16, name="et", tag="et")
                            sc = psS.tile([P, 2, 512], f32, name="sc", tag="sc")
                            nc.tensor.matmul(
                                sc[:, 0, qst:512],
                                kT_sb[0:HD, pr, ks],
                                qT_sb[0:HD, pr, qsl],
                                start=True,
                                stop=True,
                            )
                            nc.tensor.matmul(
                                sc[:, 1, qst:512],
                                kT_sb[HD:P, pr, ks],
                                qT_sb[HD:P, pr, qsl],
                                start=True,
                                stop=True,
                            )
                            nc.scalar.activation(
                                et[:, :, qst:512],
                                sc[:, :, qst:512],
                                AF.Exp,
                                scale=1.0 / 8.0,
                            )
                            if diag:
                                ms, me = M_RANGE[s]
                                for hl in range(2):
                                    nc.vector.tensor_mul(
                                        et[:, hl, ms:me],
                                        et[:, hl, ms:me],
                                        masks_sb[:, s, ms:me],
                                    )
                            nc.tensor.matmul(
                                ctx0[:],
                                va_sb[:, kb, 2 * pr, :],
                                et[:, 0, :],
                                start=(kb == 0),
                                stop=(kb == nkb - 1),
                            )
                            nc.tensor.matmul(
                                ctx1[:],
                                va_sb[:, kb, 2 * pr + 1, :],
                                et[:, 1, :],
                                start=(kb == 0),
                                stop=(kb == nkb - 1),
                            )
                        emit_normalize(ctx0, ctx1, ctxT, pr)
                    emit_outproj(ctxT, qc)

                while pending:
                    pop_one()

    nc.compile()
    return nc


def _in_maps(x, Wq, Wk, Wv, Wo, bo):
    masks = _build_masks()
    ones_r = np.ones((1, HD), dtype=np.float32)
    maps = []
    for c in range(8):
        b, g = c // 2, c % 2
        cols = slice(DG * g, DG * (g + 1))
        maps.append(
            {
                "xT": np.ascontiguousarray(np.asarray(x)[b].T, dtype=np.float32),
                "wq": np.ascontiguousarray(np.asarray(Wq)[:, cols], dtype=np.float32),
                "wk": np.ascontiguousarray(np.asarray(Wk)[:, cols], dtype=np.float32),
                "wv": np.ascontiguousarray(np.asarray(Wv)[:, cols], dtype=np.float32),
                "wo": np.ascontiguousarray(np.asarray(Wo)[cols, :], dtype=np.float32),
                "bo_b": np.broadcast_to(
                    np.asarray(bo, dtype=np.float32) / G, (P, D)
                ).copy(),
                "masks": masks,
                "ones_r": ones_r,
            }
        )
    return maps


def _get_nc():
    if "nc" not in _CACHE:
        _CACHE["nc"] = _build_bass()
    return _CACHE["nc"]


def run(inputs, trace=False):
    from concourse.bass_utils import run_bass_kernel_spmd

    nc = _get_nc()
    maps = _in_maps(**inputs)
    res = run_bass_kernel_spmd(nc, maps, list(range(8)), trace=trace)
    out = np.empty((B, S, D), dtype=np.float32)
    for c in range(8):
        b, g = c // 2, c % 2
        r = np.asarray(res.results[c]["out"])
        for qc in range(NQC):
            lo = 512 * qc + SPLIT * g
            out[b, lo : lo + SPLIT, :] = r[qc]
    return out, res


def kernel(x, Wq, Wk, Wv, Wo, bo):
    out, _ = run(dict(x=x, Wq=Wq, Wk=Wk, Wv=Wv, Wo=Wo, bo=bo))
    return out
